# revision 9
# baseline (speedup 1.0000x reference)
"""Trainium2 Bass kernel for nn_Baseline_Jinkyu (attention-LSTM collision predictor).

Strategy: batch-parallel over 8 cores (B=8, one batch element per core), with the
"V-trick": since context_t = sum_l alpha_tl * feats_tl and the LSTM input matmul is
linear, gates_ih(t) = W_ih @ context_t = sum_l alpha_tl * (W_ih[:, l*D:(l+1)*D] @ feats_tl).
V[t,l,:] = W_block_l @ feats[t,l] is precomputed OUTSIDE the sequential recurrence
(fully parallel matmuls), so the 503MB W_ih is streamed ONCE instead of T=100 times.
The recurrent loop then only needs a tiny (1,240)x(240,2048) matmul per step.
No cross-core communication at all.
"""

import os
import sys

sys.path.insert(0, "/opt/trn_rl_repo")

import numpy as np
import ml_dtypes

import concourse.bass as bass
import concourse.bacc as bacc
import concourse.tile as tile
from concourse import mybir
from concourse.bass_utils import run_bass_kernel_spmd

F32 = mybir.dt.float32
BF16 = mybir.dt.bfloat16
AF = mybir.ActivationFunctionType

B, T, L, D, H = 8, 100, 240, 256, 512
G4 = 4 * H  # 2048
NCHUNK = G4 // 512  # 4 psum chunks of 512


def _build(T=T, L=L):
    L1 = min(L, 128)
    L2 = L - L1
    nc = bacc.Bacc("TRN2", name="jinkyu", target_bir_lowering=False)

    # ---- external inputs (per core) ----
    featsA = nc.dram_tensor("featsA", [L, D, T], BF16, kind="ExternalInput")
    featsB = nc.dram_tensor("featsB", [T, D, L], F32, kind="ExternalInput")
    WT = nc.dram_tensor("WT", [L, D, G4], BF16, kind="ExternalInput")
    pwT = nc.dram_tensor("pwT", [D, D], F32, kind="ExternalInput")
    pwb = nc.dram_tensor("pwb", [128, 2], F32, kind="ExternalInput")
    wwT = nc.dram_tensor("wwT", [H, D], F32, kind="ExternalInput")
    w_b = nc.dram_tensor("w_b", [128, 2], F32, kind="ExternalInput")
    waT = nc.dram_tensor("waT", [D, 1], F32, kind="ExternalInput")
    wa_b = nc.dram_tensor("wa_b", [1, 1], F32, kind="ExternalInput")
    WhhT = nc.dram_tensor("WhhT", [H, G4], BF16, kind="ExternalInput")
    gbias = nc.dram_tensor("gbias", [128, 16], F32, kind="ExternalInput")
    m1T = nc.dram_tensor("m1T", [H, 100], F32, kind="ExternalInput")
    m1b = nc.dram_tensor("m1b", [100, 1], F32, kind="ExternalInput")
    m2T = nc.dram_tensor("m2T", [100, 50], F32, kind="ExternalInput")
    m2b = nc.dram_tensor("m2b", [50, 1], F32, kind="ExternalInput")
    m3T = nc.dram_tensor("m3T", [50, 10], F32, kind="ExternalInput")
    m3b = nc.dram_tensor("m3b", [10, 1], F32, kind="ExternalInput")
    m4T = nc.dram_tensor("m4T", [10, 2], F32, kind="ExternalInput")
    m4b = nc.dram_tensor("m4b", [2, 1], F32, kind="ExternalInput")
    eye16 = nc.dram_tensor("eye16", [16, 16], F32, kind="ExternalInput")

    # ---- outputs (per core) ----
    alphas_o = nc.dram_tensor("alphas_o", [T, L], F32, kind="ExternalOutput")
    preds_o = nc.dram_tensor("preds_o", [2, T], F32, kind="ExternalOutput")

    ts = bass.ts

    with tile.TileContext(nc) as tc:
        with tc.tile_pool(name="singles", bufs=1) as singles, \
             tc.tile_pool(name="dram", bufs=1, space="DRAM") as dram:

            # persistent weights in SBUF
            wwT_sb = singles.tile([128, 4, D], F32)
            nc.sync.dma_start(wwT_sb, wwT.rearrange("(k p) e -> p k e", p=128))
            WhhT_sb = singles.tile([128, 4, G4], BF16)
            nc.sync.dma_start(WhhT_sb, WhhT.rearrange("(k p) g -> p k g", p=128))
            pwT_sb = singles.tile([128, 2, D], F32)
            nc.sync.dma_start(pwT_sb, pwT.rearrange("(k p) e -> p k e", p=128))
            pwb_sb = singles.tile([128, 2], F32)
            nc.sync.dma_start(pwb_sb, pwb[:])
            waT_sb = singles.tile([128, 2, 1], F32)
            nc.sync.dma_start(waT_sb, waT.rearrange("(k p) o -> p k o", p=128))
            wb_sb = singles.tile([128, 2], F32)
            nc.sync.dma_start(wb_sb, w_b[:])
            wab_sb = singles.tile([1, 1], F32)
            nc.sync.dma_start(wab_sb, wa_b[:])
            gbias_sb = singles.tile([128, 16], F32)
            nc.sync.dma_start(gbias_sb, gbias[:])
            m1T_sb = singles.tile([128, 4, 100], F32)
            nc.sync.dma_start(m1T_sb, m1T.rearrange("(k p) n -> p k n", p=128))
            m1b_sb = singles.tile([100, 1], F32)
            nc.sync.dma_start(m1b_sb, m1b[:])
            m2T_sb = singles.tile([100, 50], F32)
            nc.sync.dma_start(m2T_sb, m2T[:])
            m2b_sb = singles.tile([50, 1], F32)
            nc.sync.dma_start(m2b_sb, m2b[:])
            m3T_sb = singles.tile([50, 10], F32)
            nc.sync.dma_start(m3T_sb, m3T[:])
            m3b_sb = singles.tile([10, 1], F32)
            nc.sync.dma_start(m3b_sb, m3b[:])
            m4T_sb = singles.tile([10, 2], F32)
            nc.sync.dma_start(m4T_sb, m4T[:])
            m4b_sb = singles.tile([2, 1], F32)
            nc.sync.dma_start(m4b_sb, m4b[:])
            eye16_sb = singles.tile([16, 16], F32)
            nc.sync.dma_start(eye16_sb, eye16[:])
            ones_sb = singles.tile([1, 1], F32)
            nc.vector.memset(ones_sb, 1.0)
            preds_sb = singles.tile([2, T], F32)

            # internal DRAM scratch
            proj_dram = dram.tile([T, 2, 128, L], F32)
            V_dram = dram.tile([T, L, G4], BF16)

            # ---------- phase 1: proj = feats @ pw_w.T, stored (T, e, l) ----------
            with tc.tile_pool(name="p1", bufs=3) as p1, \
                 tc.tile_pool(name="p1p", bufs=2, space="PSUM") as p1p:
                for t in range(T):
                    fB = p1.tile([128, 2, L], F32, tag="fB")
                    nc.sync.dma_start(fB, featsB[t].rearrange("(k p) l -> p k l", p=128))
                    ps = p1.tile([128, 2, L], F32, tag="ps")
                    for et in range(2):
                        pp = p1p.tile([128, L], F32, tag="pp")
                        for k in range(2):
                            nc.tensor.matmul(pp, pwT_sb[:, k, ts(et, 128)], fB[:, k, :],
                                             start=(k == 0), stop=(k == 1))
                        if et == 0:
                            nc.vector.tensor_scalar_add(ps[:, et, :], pp,
                                                        pwb_sb[:, et:et + 1])
                        else:
                            nc.scalar.activation(ps[:, et, :], pp, AF.Identity,
                                                 bias=pwb_sb[:, et:et + 1], scale=1.0)
                    nc.sync.dma_start(proj_dram[t], ps)

            # ---------- phase 2: V[t,l,:] = W_block_l^T @ feats[t,l,:] ----------
            with tc.tile_pool(name="p2", bufs=3) as p2, \
                 tc.tile_pool(name="p2p", bufs=2, space="PSUM") as p2p:
                for l in range(L):
                    wv = p2.tile([128, 2, G4], BF16, tag="wv")
                    nc.sync.dma_start(wv, WT[l].rearrange("(k p) g -> p k g", p=128))
                    fa = p2.tile([128, 2, T], BF16, tag="fa")
                    nc.sync.dma_start(fa, featsA[l].rearrange("(k p) t -> p k t", p=128))
                    vp = p2p.tile([T, G4], F32, tag="vp")
                    for k in range(2):
                        for nck in range(NCHUNK):
                            nc.tensor.matmul(vp[:, ts(nck, 512)], fa[:, k, :],
                                             wv[:, k, ts(nck, 512)],
                                             start=(k == 0), stop=(k == 1))
                    vs = p2.tile([T, G4], BF16, tag="vs")
                    nc.vector.tensor_copy(vs[:, 0:G4 // 2], vp[:, 0:G4 // 2])
                    nc.scalar.copy(vs[:, G4 // 2:G4], vp[:, G4 // 2:G4])
                    nc.sync.dma_start(V_dram[:, l, :], vs)

            # ---------- phase 3: recurrence ----------
            with tc.tile_pool(name="st", bufs=2) as st, \
                 tc.tile_pool(name="wk", bufs=2) as wk, \
                 tc.tile_pool(name="pkp", bufs=2, space="PSUM") as pkp, \
                 tc.tile_pool(name="atp", bufs=1, space="PSUM") as atp, \
                 tc.tile_pool(name="mlpp", bufs=1, space="PSUM") as mlpp, \
                 tc.tile_pool(name="gpp", bufs=1, space="PSUM") as gpp:

                hx = st.tile([128, 4], F32, tag="hx")
                nc.vector.memset(hx, 0.0)
                cx = st.tile([128, 4], F32, tag="cx")
                nc.vector.memset(cx, 0.0)
                hxbf = st.tile([128, 4], BF16, tag="hxbf")
                nc.vector.memset(hxbf, 0.0)

                for t in range(T):
                    # stream V[t] and proj[t]
                    vt0 = wk.tile([L1, G4], BF16, tag="vt0")
                    nc.sync.dma_start(vt0, V_dram[t, 0:L1, :])
                    if L2:
                        vt1 = wk.tile([L2, G4], BF16, tag="vt1")
                        nc.sync.dma_start(vt1, V_dram[t, L1:L, :])
                    pj = wk.tile([128, 2, L], F32, tag="pj")
                    nc.sync.dma_start(pj, proj_dram[t])

                    # hW = w_w @ hx  -> (256,) as two (128,1) psum cols
                    hwp = pkp.tile([128, 2], F32, tag="pk")
                    for et in range(2):
                        for k in range(4):
                            nc.tensor.matmul(hwp[:, et:et + 1],
                                             wwT_sb[:, k, ts(et, 128)],
                                             hx[:, k:k + 1],
                                             start=(k == 0), stop=(k == 3))
                    hwb = wk.tile([128, 2], F32, tag="hwb")
                    nc.vector.tensor_add(hwb, hwp, wb_sb)

                    # h_attn = tanh(proj + hW)
                    ha = wk.tile([128, 2, L], F32, tag="ha")
                    for et in range(2):
                        nc.scalar.activation(ha[:, et, :], pj[:, et, :], AF.Tanh,
                                             bias=hwb[:, et:et + 1], scale=1.0)

                    # scores = wa . h_attn -> (1, L) psum
                    scp = pkp.tile([1, L], F32, tag="pk")
                    for et in range(2):
                        nc.tensor.matmul(scp, waT_sb[:, et, :], ha[:, et, :],
                                         start=(et == 0), stop=(et == 1))

                    # softmax without max-subtraction (scores bounded by |wa|_1 ~ 4)
                    er = wk.tile([1, L], F32, tag="er")
                    nc.scalar.activation(er, scp, AF.Exp, bias=wab_sb, scale=1.0)
                    ssum = wk.tile([1, 1], F32, tag="ssum")
                    nc.vector.reduce_sum(ssum, er, axis=mybir.AxisListType.X)
                    rs = wk.tile([1, 1], F32, tag="rs")
                    nc.vector.reciprocal(rs, ssum)
                    ar = wk.tile([1, L], F32, tag="ar")
                    nc.vector.tensor_scalar_mul(ar, er, rs)
                    nc.sync.dma_start(alphas_o[t:t + 1, :], ar)

                    # transpose alpha row -> column(s), cast bf16
                    at = atp.tile([128, 2], F32, tag="at")
                    nc.tensor.matmul(at[:, 0:1], ar[:, 0:L1], ones_sb,
                                     start=True, stop=True)
                    if L2:
                        nc.tensor.matmul(at[:L2, 1:2], ar[:, L1:L], ones_sb,
                                         start=True, stop=True)
                    ab0 = wk.tile([L1, 1], BF16, tag="ab0")
                    nc.vector.tensor_copy(ab0, at[:L1, 0:1])
                    if L2:
                        ab1 = wk.tile([L2, 1], BF16, tag="ab1")
                        nc.vector.tensor_copy(ab1, at[:L2, 1:2])

                    # gates = Whh^T @ hx + alpha . V  -> psum (1, 2048)
                    gp = gpp.tile([1, G4], F32, tag="gp")
                    for nck in range(NCHUNK):
                        sl = ts(nck, 512)
                        for k in range(4):
                            nc.tensor.matmul(gp[:, sl], hxbf[:, k:k + 1],
                                             WhhT_sb[:, k, sl],
                                             start=(k == 0), stop=False)
                        nc.tensor.matmul(gp[:, sl], ab0, vt0[:, sl],
                                         start=False, stop=(L2 == 0))
                        if L2:
                            nc.tensor.matmul(gp[:, sl], ab1, vt1[:, sl],
                                             start=False, stop=True)

                    # evacuate psum row (split DVE/ACT), reshape to (16,128), transpose
                    gr = wk.tile([1, G4], F32, tag="gr")
                    nc.vector.tensor_copy(gr[:, 0:G4 // 2], gp[:, 0:G4 // 2])
                    nc.scalar.copy(gr[:, G4 // 2:G4], gp[:, G4 // 2:G4])
                    g16 = wk.tile([16, 128], F32, tag="g16")
                    nc.sync.dma_start(g16, gr)
                    gt = pkp.tile([128, 16], F32, tag="pk")
                    nc.tensor.transpose(gt, g16, eye16_sb)
                    G2 = wk.tile([128, 16], F32, tag="G2")
                    nc.vector.tensor_add(G2, gt, gbias_sb)

                    # LSTM cell (gate g = c*128+p; i: cols 0-3, f: 4-7, g: 8-11, o: 12-15)
                    ig = wk.tile([128, 4], F32, tag="ig")
                    nc.scalar.activation(ig, G2[:, 0:4], AF.Sigmoid)
                    fg = wk.tile([128, 4], F32, tag="fg")
                    nc.scalar.activation(fg, G2[:, 4:8], AF.Sigmoid)
                    gg = wk.tile([128, 4], F32, tag="gg")
                    nc.scalar.activation(gg, G2[:, 8:12], AF.Tanh)
                    og = wk.tile([128, 4], F32, tag="og")
                    nc.scalar.activation(og, G2[:, 12:16], AF.Sigmoid)
                    t1 = wk.tile([128, 4], F32, tag="t1")
                    nc.vector.tensor_mul(t1, fg, cx)
                    t2 = wk.tile([128, 4], F32, tag="t2")
                    nc.vector.tensor_mul(t2, ig, gg)
                    cx = st.tile([128, 4], F32, tag="cx")
                    nc.vector.tensor_add(cx, t1, t2)
                    th = wk.tile([128, 4], F32, tag="th")
                    nc.scalar.activation(th, cx, AF.Tanh)
                    hx = st.tile([128, 4], F32, tag="hx")
                    nc.vector.tensor_mul(hx, og, th)
                    hxbf = st.tile([128, 4], BF16, tag="hxbf")
                    nc.vector.tensor_copy(hxbf, hx)

                    # collision MLP: 512 -> 100 -> 50 -> 10 -> 2
                    x1 = mlpp.tile([100, 1], F32, tag="mlp")
                    for k in range(4):
                        nc.tensor.matmul(x1, m1T_sb[:, k, :], hx[:, k:k + 1],
                                         start=(k == 0), stop=(k == 3))
                    r1 = wk.tile([100, 1], F32, tag="r1")
                    nc.scalar.activation(r1, x1, AF.Relu, bias=m1b_sb, scale=1.0)
                    x2 = mlpp.tile([50, 1], F32, tag="mlp")
                    nc.tensor.matmul(x2, m2T_sb, r1, start=True, stop=True)
                    r2 = wk.tile([50, 1], F32, tag="r2")
                    nc.scalar.activation(r2, x2, AF.Relu, bias=m2b_sb, scale=1.0)
                    x3 = mlpp.tile([10, 1], F32, tag="mlp")
                    nc.tensor.matmul(x3, m3T_sb, r2, start=True, stop=True)
                    r3 = wk.tile([10, 1], F32, tag="r3")
                    nc.scalar.activation(r3, x3, AF.Relu, bias=m3b_sb, scale=1.0)
                    x4 = mlpp.tile([2, 1], F32, tag="mlp")
                    nc.tensor.matmul(x4, m4T_sb, r3, start=True, stop=True)
                    nc.scalar.activation(preds_sb[:, t:t + 1], x4, AF.Identity,
                                         bias=m4b_sb, scale=1.0)

                nc.sync.dma_start(preds_o[:], preds_sb)

    nc.compile()
    return nc


def _prep_shared(inputs, T_=T, L_=L):
    f32 = np.float32
    bf16 = ml_dtypes.bfloat16
    W_ih = np.asarray(inputs["W_ih"], f32)
    shared = {
        "WT": np.ascontiguousarray(
            W_ih.reshape(G4, L_, D).transpose(1, 2, 0)).astype(bf16),
        "pwT": np.ascontiguousarray(np.asarray(inputs["pw_w"], f32).T),
        "pwb": np.ascontiguousarray(np.asarray(inputs["pw_b"], f32).reshape(2, 128).T),
        "wwT": np.ascontiguousarray(np.asarray(inputs["w_w"], f32).T),  # (H, D)
        "w_b": np.ascontiguousarray(np.asarray(inputs["w_b"], f32).reshape(2, 128).T),
        "waT": np.ascontiguousarray(np.asarray(inputs["wa_w"], f32).reshape(1, D).T),
        "wa_b": np.asarray(inputs["wa_b"], f32).reshape(1, 1),
        "WhhT": np.ascontiguousarray(np.asarray(inputs["W_hh"], f32).T).astype(bf16),
        "gbias": np.ascontiguousarray(
            (np.asarray(inputs["b_ih"], f32) + np.asarray(inputs["b_hh"], f32))
            .reshape(16, 128).T),
        "m1T": np.ascontiguousarray(np.asarray(inputs["m1_w"], f32).T),
        "m1b": np.asarray(inputs["m1_b"], f32).reshape(100, 1),
        "m2T": np.ascontiguousarray(np.asarray(inputs["m2_w"], f32).T),
        "m2b": np.asarray(inputs["m2_b"], f32).reshape(50, 1),
        "m3T": np.ascontiguousarray(np.asarray(inputs["m3_w"], f32).T),
        "m3b": np.asarray(inputs["m3_b"], f32).reshape(10, 1),
        "m4T": np.ascontiguousarray(np.asarray(inputs["m4_w"], f32).T),
        "m4b": np.asarray(inputs["m4_b"], f32).reshape(2, 1),
        "eye16": np.eye(16, dtype=f32),
    }
    return shared


_last_results = None


def kernel(**inputs):
    global _last_results
    f32 = np.float32
    bf16 = ml_dtypes.bfloat16
    x = np.asarray(inputs["camera_inputs"], f32)  # (B,T,L,D)

    shared = _prep_shared(inputs)
    nc = _build()
    in_maps = []
    for b in range(B):
        xb = x[b]  # (T,L,D)
        m = dict(shared)
        m["featsA"] = np.ascontiguousarray(xb.transpose(1, 2, 0)).astype(bf16)
        m["featsB"] = np.ascontiguousarray(xb.transpose(0, 2, 1))
        in_maps.append(m)

    res = run_bass_kernel_spmd(nc, in_maps, core_ids=list(range(B)))
    _last_results = res

    preds = np.stack([res.results[b]["preds_o"].T for b in range(B)], axis=1)
    alphas = np.stack([res.results[b]["alphas_o"] for b in range(B)], axis=1)
    return preds.astype(f32), alphas.astype(f32)


# revision 13
# speedup vs baseline: 1.0486x; 1.0486x over previous
"""Trainium2 Bass kernel for nn_Baseline_Jinkyu (attention-LSTM collision predictor).

Strategy: batch-parallel over 8 cores (B=8, one batch element per core), with the
"V-trick": since context_t = sum_l alpha_tl * feats_tl and the LSTM input matmul is
linear, gates_ih(t) = W_ih @ context_t = sum_l alpha_tl * (W_ih[:, l*D:(l+1)*D] @ feats_tl).
V[t,l,:] = W_block_l @ feats[t,l] is precomputed OUTSIDE the sequential recurrence
(fully parallel matmuls), so the 503MB W_ih is streamed ONCE instead of T=100 times.
The recurrent loop then only needs a tiny (1,240)x(240,2048) matmul per step.
No cross-core communication at all.
"""

import os
import sys

sys.path.insert(0, "/opt/trn_rl_repo")

import numpy as np
import ml_dtypes

import concourse.bass as bass
import concourse.bacc as bacc
import concourse.tile as tile
from concourse import mybir
from concourse.bass_utils import run_bass_kernel_spmd

F32 = mybir.dt.float32
BF16 = mybir.dt.bfloat16
AF = mybir.ActivationFunctionType

B, T, L, D, H = 8, 100, 240, 256, 512
G4 = 4 * H  # 2048
NCHUNK = G4 // 512  # 4 psum chunks of 512


def _build(T=T, L=L, do_p1=True, do_p2=True, do_loop=True, loop_T=None):
    L1 = min(L, 128)
    L2 = L - L1
    loop_T = T if loop_T is None else loop_T
    nc = bacc.Bacc("TRN2", name="jinkyu", target_bir_lowering=False)

    # ---- external inputs (per core) ----
    featsA = nc.dram_tensor("featsA", [L, D, T], BF16, kind="ExternalInput")
    featsB = nc.dram_tensor("featsB", [T, D, L], F32, kind="ExternalInput")
    WT = nc.dram_tensor("WT", [L, D, G4], BF16, kind="ExternalInput")
    pwT = nc.dram_tensor("pwT", [D, D], F32, kind="ExternalInput")
    pwb = nc.dram_tensor("pwb", [128, 2], F32, kind="ExternalInput")
    wwT = nc.dram_tensor("wwT", [H, D], F32, kind="ExternalInput")
    w_b = nc.dram_tensor("w_b", [128, 2], F32, kind="ExternalInput")
    waT = nc.dram_tensor("waT", [D, 1], F32, kind="ExternalInput")
    wa_b = nc.dram_tensor("wa_b", [1, 1], F32, kind="ExternalInput")
    WhhT = nc.dram_tensor("WhhT", [H, G4], BF16, kind="ExternalInput")
    gbias = nc.dram_tensor("gbias", [128, 16], F32, kind="ExternalInput")
    m1T = nc.dram_tensor("m1T", [H, 100], F32, kind="ExternalInput")
    m1b = nc.dram_tensor("m1b", [100, 1], F32, kind="ExternalInput")
    m2T = nc.dram_tensor("m2T", [100, 50], F32, kind="ExternalInput")
    m2b = nc.dram_tensor("m2b", [50, 1], F32, kind="ExternalInput")
    m3T = nc.dram_tensor("m3T", [50, 10], F32, kind="ExternalInput")
    m3b = nc.dram_tensor("m3b", [10, 1], F32, kind="ExternalInput")
    m4T = nc.dram_tensor("m4T", [10, 2], F32, kind="ExternalInput")
    m4b = nc.dram_tensor("m4b", [2, 1], F32, kind="ExternalInput")
    eye16 = nc.dram_tensor("eye16", [16, 16], F32, kind="ExternalInput")

    # ---- outputs (per core) ----
    alphas_o = nc.dram_tensor("alphas_o", [T, L], F32, kind="ExternalOutput")
    preds_o = nc.dram_tensor("preds_o", [2, T], F32, kind="ExternalOutput")

    ts = bass.ts

    with tile.TileContext(nc) as tc:
        with tc.tile_pool(name="singles", bufs=1) as singles, \
             tc.tile_pool(name="dram", bufs=1, space="DRAM") as dram:

            # persistent weights in SBUF
            wwT_sb = singles.tile([128, 4, D], F32)
            nc.sync.dma_start(wwT_sb, wwT.rearrange("(k p) e -> p k e", p=128))
            WhhT_sb = singles.tile([128, 4, G4], BF16)
            nc.sync.dma_start(WhhT_sb, WhhT.rearrange("(k p) g -> p k g", p=128))
            pwT_sb = singles.tile([128, 2, D], F32)
            nc.sync.dma_start(pwT_sb, pwT.rearrange("(k p) e -> p k e", p=128))
            pwb_sb = singles.tile([128, 2], F32)
            nc.sync.dma_start(pwb_sb, pwb[:])
            waT_sb = singles.tile([128, 2, 1], F32)
            nc.sync.dma_start(waT_sb, waT.rearrange("(k p) o -> p k o", p=128))
            wb_sb = singles.tile([128, 2], F32)
            nc.sync.dma_start(wb_sb, w_b[:])
            wab_sb = singles.tile([1, 1], F32)
            nc.sync.dma_start(wab_sb, wa_b[:])
            gbias_sb = singles.tile([128, 16], F32)
            nc.sync.dma_start(gbias_sb, gbias[:])
            m1T_sb = singles.tile([128, 4, 100], F32)
            nc.sync.dma_start(m1T_sb, m1T.rearrange("(k p) n -> p k n", p=128))
            m1b_sb = singles.tile([100, 1], F32)
            nc.sync.dma_start(m1b_sb, m1b[:])
            m2T_sb = singles.tile([100, 50], F32)
            nc.sync.dma_start(m2T_sb, m2T[:])
            m2b_sb = singles.tile([50, 1], F32)
            nc.sync.dma_start(m2b_sb, m2b[:])
            m3T_sb = singles.tile([50, 10], F32)
            nc.sync.dma_start(m3T_sb, m3T[:])
            m3b_sb = singles.tile([10, 1], F32)
            nc.sync.dma_start(m3b_sb, m3b[:])
            m4T_sb = singles.tile([10, 2], F32)
            nc.sync.dma_start(m4T_sb, m4T[:])
            m4b_sb = singles.tile([2, 1], F32)
            nc.sync.dma_start(m4b_sb, m4b[:])
            eye16_sb = singles.tile([16, 16], F32)
            nc.sync.dma_start(eye16_sb, eye16[:])
            ones_sb = singles.tile([1, 1], F32)
            nc.vector.memset(ones_sb, 1.0)
            preds_sb = singles.tile([2, T], F32)

            # internal DRAM scratch
            proj_dram = dram.tile([T, 2, 128, L], F32)
            V_dram = dram.tile([T, L, G4], BF16)

            # ---------- phase 1: proj = feats @ pw_w.T, stored (T, e, l) ----------
            with tc.tile_pool(name="p1", bufs=3) as p1, \
                 tc.tile_pool(name="p1p", bufs=2, space="PSUM") as p1p:
                for t in range(T if do_p1 else 0):
                    fB = p1.tile([128, 2, L], F32, tag="fB")
                    nc.sync.dma_start(fB, featsB[t].rearrange("(k p) l -> p k l", p=128))
                    ps = p1.tile([128, 2, L], F32, tag="ps")
                    for et in range(2):
                        pp = p1p.tile([128, L], F32, tag="pp")
                        for k in range(2):
                            nc.tensor.matmul(pp, pwT_sb[:, k, ts(et, 128)], fB[:, k, :],
                                             start=(k == 0), stop=(k == 1))
                        if et == 0:
                            nc.vector.tensor_scalar_add(ps[:, et, :], pp,
                                                        pwb_sb[:, et:et + 1])
                        else:
                            nc.scalar.activation(ps[:, et, :], pp, AF.Identity,
                                                 bias=pwb_sb[:, et:et + 1], scale=1.0)
                    nc.sync.dma_start(proj_dram[t], ps)

            # ---------- phase 2: V[t,l,:] = W_block_l^T @ feats[t,l,:] ----------
            with tc.tile_pool(name="p2", bufs=3) as p2, \
                 tc.tile_pool(name="p2p", bufs=2, space="PSUM") as p2p:
                for l in range(L if do_p2 else 0):
                    wv = p2.tile([128, 2, G4], BF16, tag="wv")
                    nc.sync.dma_start(wv, WT[l].rearrange("(k p) g -> p k g", p=128))
                    fa = p2.tile([128, 2, T], BF16, tag="fa")
                    nc.sync.dma_start(fa, featsA[l].rearrange("(k p) t -> p k t", p=128))
                    vp = p2p.tile([T, G4], F32, tag="vp")
                    for k in range(2):
                        for nck in range(NCHUNK):
                            nc.tensor.matmul(vp[:, ts(nck, 512)], fa[:, k, :],
                                             wv[:, k, ts(nck, 512)],
                                             start=(k == 0), stop=(k == 1))
                    vs = p2.tile([T, G4], BF16, tag="vs")
                    nc.vector.tensor_copy(vs[:, 0:G4 // 2], vp[:, 0:G4 // 2])
                    nc.scalar.copy(vs[:, G4 // 2:G4], vp[:, G4 // 2:G4])
                    nc.sync.dma_start(V_dram[:, l, :], vs)

            # ---------- phase 3: recurrence ----------
            with tc.tile_pool(name="st", bufs=2) as st, \
                 tc.tile_pool(name="wk", bufs=2) as wk, \
                 tc.tile_pool(name="pkp", bufs=2, space="PSUM") as pkp, \
                 tc.tile_pool(name="atp", bufs=1, space="PSUM") as atp, \
                 tc.tile_pool(name="mlpp", bufs=1, space="PSUM") as mlpp, \
                 tc.tile_pool(name="gpp", bufs=1, space="PSUM") as gpp:

                hx = st.tile([128, 4], F32, tag="hx")
                nc.vector.memset(hx, 0.0)
                cx = st.tile([128, 4], F32, tag="cx")
                nc.vector.memset(cx, 0.0)
                hxbf = st.tile([128, 4], BF16, tag="hxbf")
                nc.vector.memset(hxbf, 0.0)

                for t in range(loop_T if do_loop else 0):
                    # stream V[t] and proj[t]
                    vt0 = wk.tile([L1, G4], BF16, tag="vt0")
                    nc.sync.dma_start(vt0, V_dram[t, 0:L1, :])
                    if L2:
                        vt1 = wk.tile([L2, G4], BF16, tag="vt1")
                        nc.sync.dma_start(vt1, V_dram[t, L1:L, :])
                    pj = wk.tile([128, 2, L], F32, tag="pj")
                    nc.sync.dma_start(pj, proj_dram[t])

                    # hW = w_w @ hx  -> (256,) as two (128,1) psum cols
                    hwp = pkp.tile([128, 2], F32, tag="pk")
                    for et in range(2):
                        for k in range(4):
                            nc.tensor.matmul(hwp[:, et:et + 1],
                                             wwT_sb[:, k, ts(et, 128)],
                                             hx[:, k:k + 1],
                                             start=(k == 0), stop=(k == 3))
                    hwb = wk.tile([128, 2], F32, tag="hwb")
                    nc.vector.tensor_add(hwb, hwp, wb_sb)

                    # h_attn = tanh(proj + hW)
                    ha = wk.tile([128, 2, L], F32, tag="ha")
                    for et in range(2):
                        nc.scalar.activation(ha[:, et, :], pj[:, et, :], AF.Tanh,
                                             bias=hwb[:, et:et + 1], scale=1.0)

                    # scores = wa . h_attn -> (1, L) psum
                    scp = pkp.tile([1, L], F32, tag="pk")
                    for et in range(2):
                        nc.tensor.matmul(scp, waT_sb[:, et, :], ha[:, et, :],
                                         start=(et == 0), stop=(et == 1))

                    # softmax without max-subtraction (scores bounded by |wa|_1 ~ 4)
                    er = wk.tile([1, L], F32, tag="er")
                    nc.scalar.activation(er, scp, AF.Exp, bias=wab_sb, scale=1.0)
                    ssum = wk.tile([1, 1], F32, tag="ssum")
                    nc.vector.reduce_sum(ssum, er, axis=mybir.AxisListType.X)
                    rs = wk.tile([1, 1], F32, tag="rs")
                    nc.vector.reciprocal(rs, ssum)
                    ar = wk.tile([1, L], F32, tag="ar")
                    nc.vector.tensor_scalar_mul(ar, er, rs)
                    nc.sync.dma_start(alphas_o[t:t + 1, :], ar)

                    # transpose alpha row -> column(s), cast bf16
                    at = atp.tile([128, 2], F32, tag="at")
                    nc.tensor.matmul(at[:, 0:1], ar[:, 0:L1], ones_sb,
                                     start=True, stop=True)
                    if L2:
                        nc.tensor.matmul(at[:L2, 1:2], ar[:, L1:L], ones_sb,
                                         start=True, stop=True)
                    ab0 = wk.tile([L1, 1], BF16, tag="ab0")
                    nc.vector.tensor_copy(ab0, at[:L1, 0:1])
                    if L2:
                        ab1 = wk.tile([L2, 1], BF16, tag="ab1")
                        nc.vector.tensor_copy(ab1, at[:L2, 1:2])

                    # gates = Whh^T @ hx + alpha . V  -> psum (1, 2048)
                    gp = gpp.tile([1, G4], F32, tag="gp")
                    for nck in range(NCHUNK):
                        sl = ts(nck, 512)
                        for k in range(4):
                            nc.tensor.matmul(gp[:, sl], hxbf[:, k:k + 1],
                                             WhhT_sb[:, k, sl],
                                             start=(k == 0), stop=False)
                        nc.tensor.matmul(gp[:, sl], ab0, vt0[:, sl],
                                         start=False, stop=(L2 == 0))
                        if L2:
                            nc.tensor.matmul(gp[:, sl], ab1, vt1[:, sl],
                                             start=False, stop=True)

                    # evacuate psum row (split DVE/ACT), reshape to (16,128), transpose
                    gr = wk.tile([1, G4], F32, tag="gr")
                    nc.vector.tensor_copy(gr[:, 0:G4 // 2], gp[:, 0:G4 // 2])
                    nc.scalar.copy(gr[:, G4 // 2:G4], gp[:, G4 // 2:G4])
                    g16 = wk.tile([16, 128], F32, tag="g16")
                    nc.sync.dma_start(g16, gr)
                    gt = pkp.tile([128, 16], F32, tag="pk")
                    nc.tensor.transpose(gt, g16, eye16_sb)
                    G2 = wk.tile([128, 16], F32, tag="G2")
                    nc.vector.tensor_add(G2, gt, gbias_sb)

                    # LSTM cell (gate g = c*128+p; i: cols 0-3, f: 4-7, g: 8-11, o: 12-15)
                    ig = wk.tile([128, 4], F32, tag="ig")
                    nc.scalar.activation(ig, G2[:, 0:4], AF.Sigmoid)
                    fg = wk.tile([128, 4], F32, tag="fg")
                    nc.scalar.activation(fg, G2[:, 4:8], AF.Sigmoid)
                    gg = wk.tile([128, 4], F32, tag="gg")
                    nc.scalar.activation(gg, G2[:, 8:12], AF.Tanh)
                    og = wk.tile([128, 4], F32, tag="og")
                    nc.scalar.activation(og, G2[:, 12:16], AF.Sigmoid)
                    t1 = wk.tile([128, 4], F32, tag="t1")
                    nc.vector.tensor_mul(t1, fg, cx)
                    t2 = wk.tile([128, 4], F32, tag="t2")
                    nc.vector.tensor_mul(t2, ig, gg)
                    cx = st.tile([128, 4], F32, tag="cx")
                    nc.vector.tensor_add(cx, t1, t2)
                    th = wk.tile([128, 4], F32, tag="th")
                    nc.scalar.activation(th, cx, AF.Tanh)
                    hx = st.tile([128, 4], F32, tag="hx")
                    nc.vector.tensor_mul(hx, og, th)
                    hxbf = st.tile([128, 4], BF16, tag="hxbf")
                    nc.vector.tensor_copy(hxbf, hx)

                    # collision MLP: 512 -> 100 -> 50 -> 10 -> 2
                    x1 = mlpp.tile([100, 1], F32, tag="mlp")
                    for k in range(4):
                        nc.tensor.matmul(x1, m1T_sb[:, k, :], hx[:, k:k + 1],
                                         start=(k == 0), stop=(k == 3))
                    r1 = wk.tile([100, 1], F32, tag="r1")
                    nc.scalar.activation(r1, x1, AF.Relu, bias=m1b_sb, scale=1.0)
                    x2 = mlpp.tile([50, 1], F32, tag="mlp")
                    nc.tensor.matmul(x2, m2T_sb, r1, start=True, stop=True)
                    r2 = wk.tile([50, 1], F32, tag="r2")
                    nc.scalar.activation(r2, x2, AF.Relu, bias=m2b_sb, scale=1.0)
                    x3 = mlpp.tile([10, 1], F32, tag="mlp")
                    nc.tensor.matmul(x3, m3T_sb, r2, start=True, stop=True)
                    r3 = wk.tile([10, 1], F32, tag="r3")
                    nc.scalar.activation(r3, x3, AF.Relu, bias=m3b_sb, scale=1.0)
                    x4 = mlpp.tile([2, 1], F32, tag="mlp")
                    nc.tensor.matmul(x4, m4T_sb, r3, start=True, stop=True)
                    nc.scalar.activation(preds_sb[:, t:t + 1], x4, AF.Identity,
                                         bias=m4b_sb, scale=1.0)

                nc.sync.dma_start(preds_o[:], preds_sb)

    nc.compile()
    return nc


def _prep_shared(inputs, T_=T, L_=L):
    f32 = np.float32
    bf16 = ml_dtypes.bfloat16
    W_ih = np.asarray(inputs["W_ih"], f32)
    shared = {
        "WT": np.ascontiguousarray(
            W_ih.reshape(G4, L_, D).transpose(1, 2, 0)).astype(bf16),
        "pwT": np.ascontiguousarray(np.asarray(inputs["pw_w"], f32).T),
        "pwb": np.ascontiguousarray(np.asarray(inputs["pw_b"], f32).reshape(2, 128).T),
        "wwT": np.ascontiguousarray(np.asarray(inputs["w_w"], f32).T),  # (H, D)
        "w_b": np.ascontiguousarray(np.asarray(inputs["w_b"], f32).reshape(2, 128).T),
        "waT": np.ascontiguousarray(np.asarray(inputs["wa_w"], f32).reshape(1, D).T),
        "wa_b": np.asarray(inputs["wa_b"], f32).reshape(1, 1),
        "WhhT": np.ascontiguousarray(np.asarray(inputs["W_hh"], f32).T).astype(bf16),
        "gbias": np.ascontiguousarray(
            (np.asarray(inputs["b_ih"], f32) + np.asarray(inputs["b_hh"], f32))
            .reshape(16, 128).T),
        "m1T": np.ascontiguousarray(np.asarray(inputs["m1_w"], f32).T),
        "m1b": np.asarray(inputs["m1_b"], f32).reshape(100, 1),
        "m2T": np.ascontiguousarray(np.asarray(inputs["m2_w"], f32).T),
        "m2b": np.asarray(inputs["m2_b"], f32).reshape(50, 1),
        "m3T": np.ascontiguousarray(np.asarray(inputs["m3_w"], f32).T),
        "m3b": np.asarray(inputs["m3_b"], f32).reshape(10, 1),
        "m4T": np.ascontiguousarray(np.asarray(inputs["m4_w"], f32).T),
        "m4b": np.asarray(inputs["m4_b"], f32).reshape(2, 1),
        "eye16": np.eye(16, dtype=f32),
    }
    return shared


_last_results = None


def kernel(**inputs):
    global _last_results
    f32 = np.float32
    bf16 = ml_dtypes.bfloat16
    x = np.asarray(inputs["camera_inputs"], f32)  # (B,T,L,D)

    shared = _prep_shared(inputs)
    nc = _build()
    in_maps = []
    for b in range(B):
        xb = x[b]  # (T,L,D)
        m = dict(shared)
        m["featsA"] = np.ascontiguousarray(xb.transpose(1, 2, 0)).astype(bf16)
        m["featsB"] = np.ascontiguousarray(xb.transpose(0, 2, 1))
        in_maps.append(m)

    res = run_bass_kernel_spmd(nc, in_maps, core_ids=list(range(B)))
    _last_results = res

    preds = np.stack([res.results[b]["preds_o"].T for b in range(B)], axis=1)
    alphas = np.stack([res.results[b]["alphas_o"] for b in range(B)], axis=1)
    return preds.astype(f32), alphas.astype(f32)


# revision 14
# speedup vs baseline: 1.0702x; 1.0206x over previous
"""Trainium2 Bass kernel for nn_Baseline_Jinkyu (attention-LSTM collision predictor).

Strategy: batch-parallel over 8 cores (B=8, one batch element per core), with the
"V-trick": since context_t = sum_l alpha_tl * feats_tl and the LSTM input matmul is
linear, gates_ih(t) = W_ih @ context_t = sum_l alpha_tl * (W_ih[:, l*D:(l+1)*D] @ feats_tl).
V[t,l,:] = W_block_l @ feats[t,l] is precomputed OUTSIDE the sequential recurrence
(fully parallel matmuls), so the 503MB W_ih is streamed ONCE instead of T=100 times.
The recurrent loop then only needs a tiny (1,240)x(240,2048) matmul per step.
No cross-core communication at all.
"""

import os
import sys

sys.path.insert(0, "/opt/trn_rl_repo")

import numpy as np
import ml_dtypes

import concourse.bass as bass
import concourse.bacc as bacc
import concourse.tile as tile
from concourse import mybir
from concourse.bass_utils import run_bass_kernel_spmd

F32 = mybir.dt.float32
BF16 = mybir.dt.bfloat16
AF = mybir.ActivationFunctionType

B, T, L, D, H = 8, 100, 240, 256, 512
G4 = 4 * H  # 2048
NCHUNK = G4 // 512  # 4 psum chunks of 512


def _build(T=T, L=L, do_p1=True, do_p2=True, do_loop=True, loop_T=None):
    L1 = min(L, 128)
    L2 = L - L1
    loop_T = T if loop_T is None else loop_T
    nc = bacc.Bacc("TRN2", name="jinkyu", target_bir_lowering=False)

    # ---- external inputs (per core) ----
    featsA = nc.dram_tensor("featsA", [L, D, T], BF16, kind="ExternalInput")
    featsB = nc.dram_tensor("featsB", [T, D, L], F32, kind="ExternalInput")
    WT = nc.dram_tensor("WT", [L, D, G4], BF16, kind="ExternalInput")
    pwT = nc.dram_tensor("pwT", [D, D], F32, kind="ExternalInput")
    pwb = nc.dram_tensor("pwb", [128, 2], F32, kind="ExternalInput")
    wwT = nc.dram_tensor("wwT", [H, D], F32, kind="ExternalInput")
    w_b = nc.dram_tensor("w_b", [128, 2], F32, kind="ExternalInput")
    waT = nc.dram_tensor("waT", [D, 1], F32, kind="ExternalInput")
    wa_b = nc.dram_tensor("wa_b", [1, 1], F32, kind="ExternalInput")
    WhhT = nc.dram_tensor("WhhT", [H, G4], BF16, kind="ExternalInput")
    gbias = nc.dram_tensor("gbias", [128, 16], F32, kind="ExternalInput")
    m1T = nc.dram_tensor("m1T", [H, 100], F32, kind="ExternalInput")
    m1b = nc.dram_tensor("m1b", [100, 1], F32, kind="ExternalInput")
    m2T = nc.dram_tensor("m2T", [100, 50], F32, kind="ExternalInput")
    m2b = nc.dram_tensor("m2b", [50, 1], F32, kind="ExternalInput")
    m3T = nc.dram_tensor("m3T", [50, 10], F32, kind="ExternalInput")
    m3b = nc.dram_tensor("m3b", [10, 1], F32, kind="ExternalInput")
    m4T = nc.dram_tensor("m4T", [10, 2], F32, kind="ExternalInput")
    m4b = nc.dram_tensor("m4b", [2, 1], F32, kind="ExternalInput")
    eye16 = nc.dram_tensor("eye16", [16, 16], F32, kind="ExternalInput")

    # ---- outputs (per core) ----
    alphas_o = nc.dram_tensor("alphas_o", [T, L], F32, kind="ExternalOutput")
    preds_o = nc.dram_tensor("preds_o", [2, T], F32, kind="ExternalOutput")

    ts = bass.ts

    with tile.TileContext(nc) as tc:
        with tc.tile_pool(name="singles", bufs=1) as singles, \
             tc.tile_pool(name="dram", bufs=1, space="DRAM") as dram:

            # persistent weights in SBUF
            wwT_sb = singles.tile([128, 4, D], F32)
            nc.sync.dma_start(wwT_sb, wwT.rearrange("(k p) e -> p k e", p=128))
            WhhT_sb = singles.tile([128, 4, G4], BF16)
            nc.sync.dma_start(WhhT_sb, WhhT.rearrange("(k p) g -> p k g", p=128))
            pwT_sb = singles.tile([128, 2, D], F32)
            nc.sync.dma_start(pwT_sb, pwT.rearrange("(k p) e -> p k e", p=128))
            pwb_sb = singles.tile([128, 2], F32)
            nc.sync.dma_start(pwb_sb, pwb[:])
            waT_sb = singles.tile([128, 2, 1], F32)
            nc.sync.dma_start(waT_sb, waT.rearrange("(k p) o -> p k o", p=128))
            wb_sb = singles.tile([128, 2], F32)
            nc.sync.dma_start(wb_sb, w_b[:])
            wab_sb = singles.tile([1, 1], F32)
            nc.sync.dma_start(wab_sb, wa_b[:])
            gbias_sb = singles.tile([128, 16], F32)
            nc.sync.dma_start(gbias_sb, gbias[:])
            m1T_sb = singles.tile([128, 4, 100], F32)
            nc.sync.dma_start(m1T_sb, m1T.rearrange("(k p) n -> p k n", p=128))
            m1b_sb = singles.tile([100, 1], F32)
            nc.sync.dma_start(m1b_sb, m1b[:])
            m2T_sb = singles.tile([100, 50], F32)
            nc.sync.dma_start(m2T_sb, m2T[:])
            m2b_sb = singles.tile([50, 1], F32)
            nc.sync.dma_start(m2b_sb, m2b[:])
            m3T_sb = singles.tile([50, 10], F32)
            nc.sync.dma_start(m3T_sb, m3T[:])
            m3b_sb = singles.tile([10, 1], F32)
            nc.sync.dma_start(m3b_sb, m3b[:])
            m4T_sb = singles.tile([10, 2], F32)
            nc.sync.dma_start(m4T_sb, m4T[:])
            m4b_sb = singles.tile([2, 1], F32)
            nc.sync.dma_start(m4b_sb, m4b[:])
            eye16_sb = singles.tile([16, 16], F32)
            nc.sync.dma_start(eye16_sb, eye16[:])
            ones_sb = singles.tile([1, 1], F32)
            nc.vector.memset(ones_sb, 1.0)
            preds_sb = singles.tile([2, T], F32)

            # internal DRAM scratch
            proj_dram = dram.tile([T, 2, 128, L], F32)
            V_dram = dram.tile([T, L, G4], BF16)

            # ---------- phase 1: proj = feats @ pw_w.T, stored (T, e, l) ----------
            with tc.tile_pool(name="p1", bufs=4) as p1, \
                 tc.tile_pool(name="p1p", bufs=2, space="PSUM") as p1p:
                for t in range(T if do_p1 else 0):
                    fB = p1.tile([128, 2, L], F32, tag="fB")
                    nc.sync.dma_start(fB, featsB[t].rearrange("(k p) l -> p k l", p=128))
                    ps = p1.tile([128, 2, L], F32, tag="ps")
                    for et in range(2):
                        pp = p1p.tile([128, L], F32, tag="pp")
                        for k in range(2):
                            nc.tensor.matmul(pp, pwT_sb[:, k, ts(et, 128)], fB[:, k, :],
                                             start=(k == 0), stop=(k == 1))
                        if et == 0:
                            nc.vector.tensor_scalar_add(ps[:, et, :], pp,
                                                        pwb_sb[:, et:et + 1])
                        else:
                            nc.scalar.activation(ps[:, et, :], pp, AF.Identity,
                                                 bias=pwb_sb[:, et:et + 1], scale=1.0)
                    nc.gpsimd.dma_start(proj_dram[t], ps)

            # ---------- phase 2: V[t,l,:] = W_block_l^T @ feats[t,l,:] ----------
            with tc.tile_pool(name="p2", bufs=4) as p2, \
                 tc.tile_pool(name="p2p", bufs=2, space="PSUM") as p2p:
                for l in range(L if do_p2 else 0):
                    wv0 = p2.tile([128, G4], BF16, tag="wv0")
                    nc.sync.dma_start(wv0, WT[l, 0:128, :])
                    wv1 = p2.tile([128, G4], BF16, tag="wv1")
                    nc.sync.dma_start(wv1, WT[l, 128:256, :])
                    fa = p2.tile([128, 2, T], BF16, tag="fa")
                    nc.sync.dma_start(fa, featsA[l].rearrange("(k p) t -> p k t", p=128))
                    vp = p2p.tile([T, G4], F32, tag="vp")
                    for k, wv in enumerate((wv0, wv1)):
                        for nck in range(NCHUNK):
                            nc.tensor.matmul(vp[:, ts(nck, 512)], fa[:, k, :],
                                             wv[:, ts(nck, 512)],
                                             start=(k == 0), stop=(k == 1))
                    vs = p2.tile([T, G4], BF16, tag="vs")
                    nc.vector.tensor_copy(vs[:, 0:G4 // 2], vp[:, 0:G4 // 2])
                    nc.scalar.copy(vs[:, G4 // 2:G4], vp[:, G4 // 2:G4])
                    nc.gpsimd.dma_start(V_dram[0:T // 2, l, :], vs[0:T // 2, :])
                    nc.gpsimd.dma_start(V_dram[T // 2:T, l, :], vs[T // 2:T, :])

            # ---------- phase 3: recurrence ----------
            with tc.tile_pool(name="st", bufs=2) as st, \
                 tc.tile_pool(name="wk", bufs=2) as wk, \
                 tc.tile_pool(name="pkp", bufs=2, space="PSUM") as pkp, \
                 tc.tile_pool(name="atp", bufs=1, space="PSUM") as atp, \
                 tc.tile_pool(name="mlpp", bufs=1, space="PSUM") as mlpp, \
                 tc.tile_pool(name="gpp", bufs=1, space="PSUM") as gpp:

                hx = st.tile([128, 4], F32, tag="hx")
                nc.vector.memset(hx, 0.0)
                cx = st.tile([128, 4], F32, tag="cx")
                nc.vector.memset(cx, 0.0)
                hxbf = st.tile([128, 4], BF16, tag="hxbf")
                nc.vector.memset(hxbf, 0.0)

                for t in range(loop_T if do_loop else 0):
                    # stream V[t] and proj[t]
                    vt0 = wk.tile([L1, G4], BF16, tag="vt0")
                    nc.sync.dma_start(vt0, V_dram[t, 0:L1, :])
                    if L2:
                        vt1 = wk.tile([L2, G4], BF16, tag="vt1")
                        nc.sync.dma_start(vt1, V_dram[t, L1:L, :])
                    pj = wk.tile([128, 2, L], F32, tag="pj")
                    nc.sync.dma_start(pj, proj_dram[t])

                    # hW = w_w @ hx  -> (256,) as two (128,1) psum cols
                    hwp = pkp.tile([128, 2], F32, tag="pk")
                    for et in range(2):
                        for k in range(4):
                            nc.tensor.matmul(hwp[:, et:et + 1],
                                             wwT_sb[:, k, ts(et, 128)],
                                             hx[:, k:k + 1],
                                             start=(k == 0), stop=(k == 3))
                    hwb = wk.tile([128, 2], F32, tag="hwb")
                    nc.vector.tensor_add(hwb, hwp, wb_sb)

                    # h_attn = tanh(proj + hW)
                    ha = wk.tile([128, 2, L], F32, tag="ha")
                    for et in range(2):
                        nc.scalar.activation(ha[:, et, :], pj[:, et, :], AF.Tanh,
                                             bias=hwb[:, et:et + 1], scale=1.0)

                    # scores = wa . h_attn -> (1, L) psum
                    scp = pkp.tile([1, L], F32, tag="pk")
                    for et in range(2):
                        nc.tensor.matmul(scp, waT_sb[:, et, :], ha[:, et, :],
                                         start=(et == 0), stop=(et == 1))

                    # softmax without max-subtraction (scores bounded by |wa|_1 ~ 4)
                    er = wk.tile([1, L], F32, tag="er")
                    nc.scalar.activation(er, scp, AF.Exp, bias=wab_sb, scale=1.0)
                    ssum = wk.tile([1, 1], F32, tag="ssum")
                    nc.vector.reduce_sum(ssum, er, axis=mybir.AxisListType.X)
                    rs = wk.tile([1, 1], F32, tag="rs")
                    nc.vector.reciprocal(rs, ssum)
                    ar = wk.tile([1, L], F32, tag="ar")
                    nc.vector.tensor_scalar_mul(ar, er, rs)
                    nc.sync.dma_start(alphas_o[t:t + 1, :], ar)

                    # transpose alpha row -> column(s), cast bf16
                    at = atp.tile([128, 2], F32, tag="at")
                    nc.tensor.matmul(at[:, 0:1], ar[:, 0:L1], ones_sb,
                                     start=True, stop=True)
                    if L2:
                        nc.tensor.matmul(at[:L2, 1:2], ar[:, L1:L], ones_sb,
                                         start=True, stop=True)
                    ab0 = wk.tile([L1, 1], BF16, tag="ab0")
                    nc.vector.tensor_copy(ab0, at[:L1, 0:1])
                    if L2:
                        ab1 = wk.tile([L2, 1], BF16, tag="ab1")
                        nc.vector.tensor_copy(ab1, at[:L2, 1:2])

                    # gates = Whh^T @ hx + alpha . V  -> psum (1, 2048)
                    gp = gpp.tile([1, G4], F32, tag="gp")
                    for nck in range(NCHUNK):
                        sl = ts(nck, 512)
                        for k in range(4):
                            nc.tensor.matmul(gp[:, sl], hxbf[:, k:k + 1],
                                             WhhT_sb[:, k, sl],
                                             start=(k == 0), stop=False)
                        nc.tensor.matmul(gp[:, sl], ab0, vt0[:, sl],
                                         start=False, stop=(L2 == 0))
                        if L2:
                            nc.tensor.matmul(gp[:, sl], ab1, vt1[:, sl],
                                             start=False, stop=True)

                    # evacuate psum row (split DVE/ACT), reshape to (16,128), transpose
                    gr = wk.tile([1, G4], F32, tag="gr")
                    nc.vector.tensor_copy(gr[:, 0:G4 // 2], gp[:, 0:G4 // 2])
                    nc.scalar.copy(gr[:, G4 // 2:G4], gp[:, G4 // 2:G4])
                    g16 = wk.tile([16, 128], F32, tag="g16")
                    nc.sync.dma_start(g16, gr)
                    gt = pkp.tile([128, 16], F32, tag="pk")
                    nc.tensor.transpose(gt, g16, eye16_sb)
                    G2 = wk.tile([128, 16], F32, tag="G2")
                    nc.vector.tensor_add(G2, gt, gbias_sb)

                    # LSTM cell (gate g = c*128+p; i: cols 0-3, f: 4-7, g: 8-11, o: 12-15)
                    ig = wk.tile([128, 4], F32, tag="ig")
                    nc.scalar.activation(ig, G2[:, 0:4], AF.Sigmoid)
                    fg = wk.tile([128, 4], F32, tag="fg")
                    nc.scalar.activation(fg, G2[:, 4:8], AF.Sigmoid)
                    gg = wk.tile([128, 4], F32, tag="gg")
                    nc.scalar.activation(gg, G2[:, 8:12], AF.Tanh)
                    og = wk.tile([128, 4], F32, tag="og")
                    nc.scalar.activation(og, G2[:, 12:16], AF.Sigmoid)
                    t1 = wk.tile([128, 4], F32, tag="t1")
                    nc.vector.tensor_mul(t1, fg, cx)
                    t2 = wk.tile([128, 4], F32, tag="t2")
                    nc.vector.tensor_mul(t2, ig, gg)
                    cx = st.tile([128, 4], F32, tag="cx")
                    nc.vector.tensor_add(cx, t1, t2)
                    th = wk.tile([128, 4], F32, tag="th")
                    nc.scalar.activation(th, cx, AF.Tanh)
                    hx = st.tile([128, 4], F32, tag="hx")
                    nc.vector.tensor_mul(hx, og, th)
                    hxbf = st.tile([128, 4], BF16, tag="hxbf")
                    nc.vector.tensor_copy(hxbf, hx)

                    # collision MLP: 512 -> 100 -> 50 -> 10 -> 2
                    x1 = mlpp.tile([100, 1], F32, tag="mlp")
                    for k in range(4):
                        nc.tensor.matmul(x1, m1T_sb[:, k, :], hx[:, k:k + 1],
                                         start=(k == 0), stop=(k == 3))
                    r1 = wk.tile([100, 1], F32, tag="r1")
                    nc.scalar.activation(r1, x1, AF.Relu, bias=m1b_sb, scale=1.0)
                    x2 = mlpp.tile([50, 1], F32, tag="mlp")
                    nc.tensor.matmul(x2, m2T_sb, r1, start=True, stop=True)
                    r2 = wk.tile([50, 1], F32, tag="r2")
                    nc.scalar.activation(r2, x2, AF.Relu, bias=m2b_sb, scale=1.0)
                    x3 = mlpp.tile([10, 1], F32, tag="mlp")
                    nc.tensor.matmul(x3, m3T_sb, r2, start=True, stop=True)
                    r3 = wk.tile([10, 1], F32, tag="r3")
                    nc.scalar.activation(r3, x3, AF.Relu, bias=m3b_sb, scale=1.0)
                    x4 = mlpp.tile([2, 1], F32, tag="mlp")
                    nc.tensor.matmul(x4, m4T_sb, r3, start=True, stop=True)
                    nc.scalar.activation(preds_sb[:, t:t + 1], x4, AF.Identity,
                                         bias=m4b_sb, scale=1.0)

                nc.sync.dma_start(preds_o[:], preds_sb)

    nc.compile()
    return nc


def _prep_shared(inputs, T_=T, L_=L):
    f32 = np.float32
    bf16 = ml_dtypes.bfloat16
    W_ih = np.asarray(inputs["W_ih"], f32)
    shared = {
        "WT": np.ascontiguousarray(
            W_ih.reshape(G4, L_, D).transpose(1, 2, 0)).astype(bf16),
        "pwT": np.ascontiguousarray(np.asarray(inputs["pw_w"], f32).T),
        "pwb": np.ascontiguousarray(np.asarray(inputs["pw_b"], f32).reshape(2, 128).T),
        "wwT": np.ascontiguousarray(np.asarray(inputs["w_w"], f32).T),  # (H, D)
        "w_b": np.ascontiguousarray(np.asarray(inputs["w_b"], f32).reshape(2, 128).T),
        "waT": np.ascontiguousarray(np.asarray(inputs["wa_w"], f32).reshape(1, D).T),
        "wa_b": np.asarray(inputs["wa_b"], f32).reshape(1, 1),
        "WhhT": np.ascontiguousarray(np.asarray(inputs["W_hh"], f32).T).astype(bf16),
        "gbias": np.ascontiguousarray(
            (np.asarray(inputs["b_ih"], f32) + np.asarray(inputs["b_hh"], f32))
            .reshape(16, 128).T),
        "m1T": np.ascontiguousarray(np.asarray(inputs["m1_w"], f32).T),
        "m1b": np.asarray(inputs["m1_b"], f32).reshape(100, 1),
        "m2T": np.ascontiguousarray(np.asarray(inputs["m2_w"], f32).T),
        "m2b": np.asarray(inputs["m2_b"], f32).reshape(50, 1),
        "m3T": np.ascontiguousarray(np.asarray(inputs["m3_w"], f32).T),
        "m3b": np.asarray(inputs["m3_b"], f32).reshape(10, 1),
        "m4T": np.ascontiguousarray(np.asarray(inputs["m4_w"], f32).T),
        "m4b": np.asarray(inputs["m4_b"], f32).reshape(2, 1),
        "eye16": np.eye(16, dtype=f32),
    }
    return shared


_last_results = None


def kernel(**inputs):
    global _last_results
    f32 = np.float32
    bf16 = ml_dtypes.bfloat16
    x = np.asarray(inputs["camera_inputs"], f32)  # (B,T,L,D)

    shared = _prep_shared(inputs)
    nc = _build()
    in_maps = []
    for b in range(B):
        xb = x[b]  # (T,L,D)
        m = dict(shared)
        m["featsA"] = np.ascontiguousarray(xb.transpose(1, 2, 0)).astype(bf16)
        m["featsB"] = np.ascontiguousarray(xb.transpose(0, 2, 1))
        in_maps.append(m)

    res = run_bass_kernel_spmd(nc, in_maps, core_ids=list(range(B)))
    _last_results = res

    preds = np.stack([res.results[b]["preds_o"].T for b in range(B)], axis=1)
    alphas = np.stack([res.results[b]["alphas_o"] for b in range(B)], axis=1)
    return preds.astype(f32), alphas.astype(f32)
